# revision 7
# baseline (speedup 1.0000x reference)
"""RandomGraphMixer3D Trainium2 kernel.

Math (per batch b):
    out[b, o, n] = bias[o] + sum_{c, r} W[o, c, r] * x[b, c, idx[n, r]]

Strategy (8 NeuronCores, node-sharded):
  - Host reshapes x to x_t[n, (b, c)] bf16 (rows of 512 bf16 = 1KB).
  - Each core owns a 4096-node slice of the output (all 16 batches).
  - Device gathers x_t rows for its (n, r) pairs with dma_gather
    (transpose=True): payload element (b*32+c) lands on partition
    (b%4)*32+c, free chunk k=b//4.  One gathered column therefore holds
    4 batches x 32 channels.
  - PE contracts with a block-diagonal lhsT (4 copies of W_r^T on the
    diagonal), accumulating the 16 r-terms in PSUM.  Pair order is
    (n-slice, r, n) so each r-block of 512 gathered columns aligns with
    the same 512 output nodes.
  - DVE adds bias while evacuating PSUM, HWDGE stores f32 output.
"""

import numpy as np
import ml_dtypes

import concourse.bacc as bacc
import concourse.mybir as mybir
from concourse import tile
from concourse.bass_utils import run_bass_kernel_spmd

BT, C, N, R, CO = 16, 32, 32768, 16, 32
NCORES = 8
NS = N // NCORES          # nodes per core = 4096
SN = 512                  # nodes per j-slice (psum free dim)
JS = NS // SN             # j-slices per core = 8
CH = SN * R               # pairs per gather call = 8192
PAIRS = NS * R            # pairs per core = 65536
KQ = BT // 4              # batch quads = 4

_CACHE = {}


def _build_nc():
    nc = bacc.Bacc(None, target_bir_lowering=False, num_swdge_queues=4)
    f32 = mybir.dt.float32
    bf16 = mybir.dt.bfloat16

    xt = nc.dram_tensor("xt", [N, BT * C], bf16, kind="ExternalInput")
    idx = nc.dram_tensor("idx", [128, PAIRS // 16], mybir.dt.int16,
                         kind="ExternalInput")
    wbd = nc.dram_tensor("wbd", [128, R * 128], bf16, kind="ExternalInput")
    bias = nc.dram_tensor("bias", [128, 1], f32, kind="ExternalInput")
    out = nc.dram_tensor("out", [BT * CO, NS], f32, kind="ExternalOutput")

    with tile.TileContext(nc) as tc:
        with (
            tc.tile_pool(name="const", bufs=1) as constp,
            tc.tile_pool(name="gat", bufs=12) as gatp,
            tc.tile_pool(name="evac", bufs=3) as evacp,
            tc.tile_pool(name="ps", bufs=2, space="PSUM") as psp,
        ):
            idx_sb = constp.tile([128, PAIRS // 16], mybir.dt.int16)
            nc.sync.dma_start(idx_sb[:], idx[:])
            w_sb = constp.tile([128, R * 128], bf16)
            nc.sync.dma_start(w_sb[:], wbd[:])
            b_sb = constp.tile([128, 1], f32)
            nc.sync.dma_start(b_sb[:], bias[:])

            qn = 0
            for j in range(JS):
                # One gather per (j, r): 512 idxs -> 67 s2m descriptors,
                # under the 128-entry SWDGE ring FIFO limit per queue.
                gs = []
                for r in range(R):
                    g = gatp.tile([128, KQ, SN], bf16, tag="g",
                                  name=f"g_{j}_{r}")
                    col0 = (j * R + r) * (SN // 16)
                    nc.gpsimd.dma_gather(
                        g[:, :, :],
                        xt[:],
                        idx_sb[:, col0:col0 + SN // 16],
                        SN,
                        SN,
                        BT * C,
                        transpose=True,
                        queue_num=qn % 4,
                    )
                    qn += 1
                    gs.append(g)
                pss = [psp.tile([128, SN], f32, tag=f"psum{k}",
                                name=f"ps_{j}_{k}")
                       for k in range(KQ)]
                for r in range(R):
                    lhsT = w_sb[:, r * 128:(r + 1) * 128]
                    for k in range(KQ):
                        nc.tensor.matmul(
                            pss[k][:],
                            lhsT,
                            gs[r][:, k, :],
                            start=(r == 0),
                            stop=(r == R - 1),
                        )
                for k in range(KQ):
                    ot = evacp.tile([128, SN], f32, tag="o")
                    nc.vector.tensor_add(
                        ot[:], pss[k][:], b_sb[:].to_broadcast([128, SN]))
                    nc.sync.dma_start(
                        out[k * 128:(k + 1) * 128, j * SN:(j + 1) * SN],
                        ot[:],
                    )
    nc.compile()
    return nc


def _get_nc():
    if "nc" not in _CACHE:
        _CACHE["nc"] = _build_nc()
    return _CACHE["nc"]


def _host_prep(x, weight, bias, rand_indices):
    x = np.asarray(x, dtype=np.float32)
    weight = np.asarray(weight, dtype=np.float32)
    bias = np.asarray(bias, dtype=np.float32)
    rand_indices = np.asarray(rand_indices)

    # x_t[n, b*32+c] = x[b, c, n]
    x_t = np.ascontiguousarray(
        x.reshape(BT, C, N).transpose(2, 0, 1).reshape(N, BT * C)
    ).astype(ml_dtypes.bfloat16)

    # Block-diagonal lhsT per r: wbd[b4*32+c, r*128 + b4*32+o] = W[o, c, r]
    wbd = np.zeros((128, R * 128), np.float32)
    for r in range(R):
        wt = weight[:, :, r].T  # [c, o]
        for b4 in range(4):
            wbd[b4 * 32:(b4 + 1) * 32,
                r * 128 + b4 * 32: r * 128 + (b4 + 1) * 32] = wt
    wbd = wbd.astype(ml_dtypes.bfloat16)

    bias_col = np.ascontiguousarray(
        np.tile(bias, 4).reshape(128, 1).astype(np.float32))

    in_maps = []
    for s in range(NCORES):
        ri = rand_indices[s * NS:(s + 1) * NS].astype(np.int16)  # [NS, R]
        # pair order (j, r, nn): ia[j*CH + r*SN + nn] = ri[j*SN + nn, r]
        ia = ri.reshape(JS, SN, R).transpose(0, 2, 1).reshape(PAIRS)
        wrapped = ia.reshape(PAIRS // 16, 16).T          # [16, PAIRS//16]
        idx128 = np.ascontiguousarray(np.tile(wrapped, (8, 1)))
        in_maps.append({
            "xt": x_t,
            "idx": idx128,
            "wbd": wbd,
            "bias": bias_col,
        })
    return in_maps


def _run(x, weight, bias, rand_indices, trace=False):
    nc = _get_nc()
    in_maps = _host_prep(x, weight, bias, rand_indices)
    res = run_bass_kernel_spmd(
        nc, in_maps, core_ids=list(range(NCORES)), trace=trace)
    out = np.empty((BT, CO, N), np.float32)
    for s in range(NCORES):
        out[:, :, s * NS:(s + 1) * NS] = (
            res.results[s]["out"].reshape(BT, CO, NS))
    return out.reshape(BT, CO, 32, 32, 32), res.exec_time_ns


def kernel(x, weight, bias, rand_indices):
    out, _ = _run(x, weight, bias, rand_indices)
    return out


# revision 9
# speedup vs baseline: 1.0618x; 1.0618x over previous
"""RandomGraphMixer3D Trainium2 kernel.

Math (per batch b):
    out[b, o, n] = bias[o] + sum_{c, r} W[o, c, r] * x[b, c, idx[n, r]]

Strategy (8 NeuronCores, node-sharded):
  - Host reshapes x to x_t[n, (b, c)] bf16 (rows of 512 bf16 = 1KB).
  - Each core owns a 4096-node slice of the output (all 16 batches).
  - Device gathers x_t rows for its (n, r) pairs with dma_gather
    (transpose=True): payload element (b*32+c) lands on partition
    (b%4)*32+c, free chunk k=b//4.  One gathered column therefore holds
    4 batches x 32 channels.
  - PE contracts with a block-diagonal lhsT (4 copies of W_r^T on the
    diagonal), accumulating the 16 r-terms in PSUM.  Pair order is
    (n-slice, r, n) so each r-block of 512 gathered columns aligns with
    the same 512 output nodes.
  - DVE adds bias while evacuating PSUM, HWDGE stores f32 output.
"""

import numpy as np
import ml_dtypes

import concourse.bacc as bacc
import concourse.mybir as mybir
from concourse import tile
from concourse.bass_utils import run_bass_kernel_spmd

BT, C, N, R, CO = 16, 32, 32768, 16, 32
NCORES = 8
NS = N // NCORES          # nodes per core = 4096
SN = 512                  # nodes per j-slice (psum free dim)
JS = NS // SN             # j-slices per core = 8
CH = SN * R               # pairs per gather call = 8192
PAIRS = NS * R            # pairs per core = 65536
KQ = BT // 4              # batch quads = 4

_CACHE = {}


def _build_nc():
    nc = bacc.Bacc(None, target_bir_lowering=False, num_swdge_queues=4)
    f32 = mybir.dt.float32
    bf16 = mybir.dt.bfloat16

    xt = nc.dram_tensor("xt", [N, BT * C], bf16, kind="ExternalInput")
    idx = nc.dram_tensor("idx", [128, PAIRS // 16], mybir.dt.int16,
                         kind="ExternalInput")
    wbd = nc.dram_tensor("wbd", [128, R * 128], bf16, kind="ExternalInput")
    bias = nc.dram_tensor("bias", [128, 1], f32, kind="ExternalInput")
    out = nc.dram_tensor("out", [BT * CO, NS], f32, kind="ExternalOutput")

    with tile.TileContext(nc) as tc:
        with (
            tc.tile_pool(name="const", bufs=1) as constp,
            tc.tile_pool(name="gat", bufs=24) as gatp,
            tc.tile_pool(name="evac", bufs=6) as evacp,
            tc.tile_pool(name="ps", bufs=2, space="PSUM") as psp,
        ):
            # Split the idx load so the first gathers start immediately.
            J0C = CH // 16  # idx columns per j-slice
            idx_sb = constp.tile([128, PAIRS // 16], mybir.dt.int16)
            nc.sync.dma_start(idx_sb[:, :J0C], idx[:, :J0C])
            w_sb = constp.tile([128, R * 128], bf16)
            nc.sync.dma_start(w_sb[:], wbd[:])
            b_sb = constp.tile([128, 1], f32)
            nc.sync.dma_start(b_sb[:], bias[:])
            nc.sync.dma_start(idx_sb[:, J0C:], idx[:, J0C:])

            qn = 0
            for j in range(JS):
                # One gather per (j, r): 512 idxs -> 67 s2m descriptors,
                # under the 128-entry SWDGE ring FIFO limit per queue.
                gs = []
                for r in range(R):
                    g = gatp.tile([128, KQ, SN], bf16, tag="g",
                                  name=f"g_{j}_{r}")
                    col0 = (j * R + r) * (SN // 16)
                    nc.gpsimd.dma_gather(
                        g[:, :, :],
                        xt[:],
                        idx_sb[:, col0:col0 + SN // 16],
                        SN,
                        SN,
                        BT * C,
                        transpose=True,
                        queue_num=qn % 4,
                    )
                    qn += 1
                    gs.append(g)
                pss = [psp.tile([128, SN], f32, tag=f"psum{k}",
                                name=f"ps_{j}_{k}")
                       for k in range(KQ)]
                for r in range(R):
                    lhsT = w_sb[:, r * 128:(r + 1) * 128]
                    for k in range(KQ):
                        nc.tensor.matmul(
                            pss[k][:],
                            lhsT,
                            gs[r][:, k, :],
                            start=(r == 0),
                            stop=(r == R - 1),
                        )
                for k in range(KQ):
                    ot = evacp.tile([128, SN], f32, tag="o")
                    nc.vector.tensor_add(
                        ot[:], pss[k][:], b_sb[:].to_broadcast([128, SN]))
                    nc.sync.dma_start(
                        out[k * 128:(k + 1) * 128, j * SN:(j + 1) * SN],
                        ot[:],
                    )
    nc.compile()
    return nc


def _get_nc():
    if "nc" not in _CACHE:
        _CACHE["nc"] = _build_nc()
    return _CACHE["nc"]


def _host_prep(x, weight, bias, rand_indices):
    x = np.asarray(x, dtype=np.float32)
    weight = np.asarray(weight, dtype=np.float32)
    bias = np.asarray(bias, dtype=np.float32)
    rand_indices = np.asarray(rand_indices)

    # x_t[n, b*32+c] = x[b, c, n]
    x_t = np.ascontiguousarray(
        x.reshape(BT, C, N).transpose(2, 0, 1).reshape(N, BT * C)
    ).astype(ml_dtypes.bfloat16)

    # Block-diagonal lhsT per r: wbd[b4*32+c, r*128 + b4*32+o] = W[o, c, r]
    wbd = np.zeros((128, R * 128), np.float32)
    for r in range(R):
        wt = weight[:, :, r].T  # [c, o]
        for b4 in range(4):
            wbd[b4 * 32:(b4 + 1) * 32,
                r * 128 + b4 * 32: r * 128 + (b4 + 1) * 32] = wt
    wbd = wbd.astype(ml_dtypes.bfloat16)

    bias_col = np.ascontiguousarray(
        np.tile(bias, 4).reshape(128, 1).astype(np.float32))

    in_maps = []
    for s in range(NCORES):
        ri = rand_indices[s * NS:(s + 1) * NS].astype(np.int16)  # [NS, R]
        # pair order (j, r, nn): ia[j*CH + r*SN + nn] = ri[j*SN + nn, r]
        ia = ri.reshape(JS, SN, R).transpose(0, 2, 1).reshape(PAIRS)
        wrapped = ia.reshape(PAIRS // 16, 16).T          # [16, PAIRS//16]
        idx128 = np.ascontiguousarray(np.tile(wrapped, (8, 1)))
        in_maps.append({
            "xt": x_t,
            "idx": idx128,
            "wbd": wbd,
            "bias": bias_col,
        })
    return in_maps


def _run(x, weight, bias, rand_indices, trace=False):
    nc = _get_nc()
    in_maps = _host_prep(x, weight, bias, rand_indices)
    res = run_bass_kernel_spmd(
        nc, in_maps, core_ids=list(range(NCORES)), trace=trace)
    out = np.empty((BT, CO, N), np.float32)
    for s in range(NCORES):
        out[:, :, s * NS:(s + 1) * NS] = (
            res.results[s]["out"].reshape(BT, CO, NS))
    return out.reshape(BT, CO, 32, 32, 32), res.exec_time_ns


def kernel(x, weight, bias, rand_indices):
    out, _ = _run(x, weight, bias, rand_indices)
    return out
